# revision 4
# baseline (speedup 1.0000x reference)
"""Cross-attention Trainium2 Bass kernel (fp8 DoubleRow + DMA-transpose).

Problem (per batch element, fp32 inputs):
    q = x1 @ Wq + bq; k = x2 @ Wk + bk; v = x2 @ Wv + bv
    out = softmax(q k^T / sqrt(512)) @ v        with LQ = LK = 2048, D = 512

Sharding: batch (B=8) across the 8 NeuronCores, one batch element per core;
weights replicated. Full inputs in, full output out.

Per-core plan:
  - X^T via DMA-XBAR transposes (bf16), casts on DVE. No PE transposes.
  - Q/K projections and the scores matmul run in fp8e4 with
    MatmulPerfMode.DoubleRow (2 contraction rows/partition, 0.5 cyc/row):
    weights pre-scaled by 32 (host) so W values sit in e4m3 normal range;
    exp scale folds the 1/(32*32) back out. Softmax damps the fp8
    relative error, so only the q/k path can run this hot.
  - V projection, P (=exp) tiles, and the P@V matmul stay bf16 (full
    1 cyc/row on PE, precision safe for the 2e-2 gate).
  - softmax denominators via ones[128,2] @ P^T matmuls (bf16) accumulated
    over k-tiles; DRAM bounce turns the row into per-partition columns.
"""
import sys

sys.path.insert(0, "/opt/trn_rl_repo")
import numpy as np
import concourse.bass as bass
import concourse.tile as tile
from concourse.tile import add_dep_helper
import concourse.bacc as bacc
from concourse import mybir
from concourse.bass_utils import run_bass_kernel_spmd

B, LQ, LK, D = 8, 2048, 2048, 512
P = 128
NKT = LK // P          # 16 k-tiles
NDC = D // P           # 4 d-chunks
NQB = LQ // 512        # 4 q-blocks of 512
NCORES = 8
W8S = 32.0             # fp8 pre-scale on Wq/Wk (and bq/bk)
SCALE = float(1.0 / (np.sqrt(np.float32(D)) * W8S * W8S))

f32 = mybir.dt.float32
f32r = mybir.dt.float32r
bf16 = mybir.dt.bfloat16
f8 = mybir.dt.float8e4
ts = bass.ts
Exp = mybir.ActivationFunctionType.Exp
DR = mybir.MatmulPerfMode.DoubleRow

_CACHE = {}


def _build():
    nc = bacc.Bacc("TRN2", target_bir_lowering=False, debug=False,
                   num_devices=NCORES)
    X1 = nc.declare_dram_parameter("x1", [LQ, D], f32, isOutput=False)
    X2 = nc.declare_dram_parameter("x2", [LK, D], f32, isOutput=False)
    WQ = nc.declare_dram_parameter("wq", [D, D], f32, isOutput=False)
    WK = nc.declare_dram_parameter("wk", [D, D], f32, isOutput=False)
    WV = nc.declare_dram_parameter("wv", [D, D], f32, isOutput=False)
    BV = nc.declare_dram_parameter("bv", [D], f32, isOutput=False)
    BP = nc.declare_dram_parameter("bpack", [P, 8], f32, isOutput=False)
    OUT = nc.declare_dram_parameter("out", [LQ, D], f32, isOutput=True)
    DEN = nc.dram_tensor("den_scratch", [NQB, 512], f32)

    with tile.TileContext(nc) as tc:
        with (
            tc.tile_pool(name="const", bufs=1) as cpool,
            tc.tile_pool(name="wts", bufs=1) as wpool,
            tc.tile_pool(name="stage", bufs=2) as stage,
            tc.tile_pool(name="wstage", bufs=3) as wstage,
            tc.tile_pool(name="xbt", bufs=2) as xbt,
            tc.tile_pool(name="xtp", bufs=2) as xtp,
            tc.tile_pool(name="x8p", bufs=2) as x8p,
            tc.tile_pool(name="q8p", bufs=2) as q8p,
            tc.tile_pool(name="big", bufs=1) as big,
            tc.tile_pool(name="ptp", bufs=16) as ptp,
            tc.tile_pool(name="obuf", bufs=2) as obuf,
            tc.tile_pool(name="psV", bufs=2, space="PSUM") as psV,
            tc.tile_pool(name="psS", bufs=2, space="PSUM") as psS,
            tc.tile_pool(name="psO", bufs=3, space="PSUM") as psO,
            tc.tile_pool(name="psD", bufs=1, space="PSUM") as psD,
        ):
            dma_insts = {}

            def load_x_block(X, blk, qsel, after=None):
                """One 1MB DMA: rows blk*512..+512 as [128, 4, 512]."""
                xin = stage.tile([P, 4, D], f32, tag="xin",
                                 name=f"xin_{qsel}_{blk}")
                src = X.ap().rearrange("(b t p) d -> b p t d", p=P, t=4)[blk]
                eng = nc.sync if (blk + qsel) % 2 == 0 else nc.scalar
                di = eng.dma_start(xin[:], src)
                if after is not None:
                    add_dep_helper(di.ins, dma_insts[after].ins,
                                   reason="stagger DMA bandwidth")
                dma_insts[f"x{qsel}_{blk}"] = di
                return xin

            def load_w(W, name, qsel, after=None):
                wst = wstage.tile([P, 4, D], f32, tag="wst",
                                  name=f"wst_{name}")
                src = W.ap().rearrange("(c p) n -> p c n", p=P)
                eng = nc.sync if qsel % 2 == 0 else nc.scalar
                di = eng.dma_start(wst[:], src)
                if after is not None:
                    add_dep_helper(di.ins, dma_insts[after].ins,
                                   reason="stagger DMA bandwidth")
                dma_insts[name] = di
                return wst

            # ---- weight DMAs up front (wv first: V proj is first PE work) --
            wst_v = load_w(WV, "wv", 1)
            xin2_0 = load_x_block(X2, 0, 0)
            wst_k = load_w(WK, "wk", 0, after="x0_0")
            wst_q = load_w(WQ, "wq", 1, after="wv")

            wv_b = wpool.tile([P, 4, D], bf16, tag="wv_b", name="wv_b")
            nc.vector.tensor_copy(wv_b[:], wst_v[:])
            # fp8 weights, pre-scaled by 32: w8x[j][p, i, n] = 32*W[j*256+i*128+p, n]
            w8k = [wpool.tile([P, 2, D], f8, tag=f"w8k{j}", name=f"w8k{j}")
                   for j in range(2)]
            w8q = [wpool.tile([P, 2, D], f8, tag=f"w8q{j}", name=f"w8q{j}")
                   for j in range(2)]
            for j in range(2):
                nc.vector.tensor_scalar_mul(w8k[j][:], wst_k[:, 2 * j:2 * j + 2, :], W8S)
            for j in range(2):
                nc.vector.tensor_scalar_mul(w8q[j][:], wst_q[:, 2 * j:2 * j + 2, :], W8S)

            # persistent K^T (fp8 packed) and V (bf16)
            k8 = [big.tile([P, 2, LK], f8, tag=f"k8_{j}", name=f"k8_{j}")
                  for j in range(2)]
            vt = [big.tile([P, D], bf16, tag=f"v{t}", name=f"v{t}")
                  for t in range(NKT)]

            # ---- small constants ----
            ones2_b = cpool.tile([P, 2], bf16, tag="ones2_b")
            nc.vector.memset(ones2_b[:], 1.0)

            bv_f = cpool.tile([1, D], f32, tag="bv_f")
            nc.scalar.dma_start(bv_f[:], BV[:].unsqueeze(0))
            onesr_f = cpool.tile([1, P], f32, tag="onesr_f")
            nc.vector.memset(onesr_f[:], 1.0)
            ones_row = cpool.tile([1, P], f32r, tag="ones_row")
            nc.vector.tensor_copy(ones_row[:], onesr_f[:])
            bv_row = cpool.tile([1, D], f32r, tag="bv_row")
            nc.vector.tensor_copy(bv_row[:], bv_f[:])
            bvb_ps = psV.tile([P, D], f32, tag="pv", name="bvb_ps")
            nc.tensor.matmul(bvb_ps[:], ones_row[:], bv_row[:],
                             start=True, stop=True)
            bv_bcast = cpool.tile([P, D], f32, tag="bv_bcast")
            nc.vector.tensor_copy(bv_bcast[:], bvb_ps[:])

            # per-partition bias columns (32*bq at 0-3, 32*bk at 4-7)
            bpack = cpool.tile([P, 8], f32, tag="bpack")
            nc.scalar.dma_start(bpack[:], BP[:])
            bq_t = [bpack[:, ci:ci + 1] for ci in range(NDC)]
            bk_t = [bpack[:, 4 + ci:5 + ci] for ci in range(NDC)]

            def transpose_block(xin, qsel, blk):
                """f32 [128,4,512] block -> bf16 X^T tile [128, 4, 512]
                ([p, c, s] = X^T[c*128+p, blk*512+s]) via DVE cast + 4
                DMA-XBAR transposes; plus fp8 pack tiles x8[j]=[128,2,512]."""
                xb = xbt.tile([P, 4, D], bf16, tag="xb",
                              name=f"xb_{qsel}_{blk}")
                xt = xtp.tile([P, 4, D], bf16, tag=f"xt{qsel}",
                              name=f"xt{qsel}_{blk}")
                for tp in range(4):
                    nc.vector.tensor_copy(xb[:, tp, :], xin[:, tp, :])
                    eng = nc.sync if (tp + blk) % 2 == 0 else nc.scalar
                    eng.dma_start_transpose(xt[:, :, ts(tp, P)], xb[:, tp, :])
                x8 = [x8p.tile([P, 2, D], f8, tag=f"x8{qsel}_{j}",
                               name=f"x8{qsel}_{j}_{blk}") for j in range(2)]
                for j in range(2):
                    nc.vector.tensor_copy(x8[j][:], xt[:, 2 * j:2 * j + 2, :])
                return xt, x8

            # ---------------- phase A1: X2 -> K^T (fp8), V (bf16) ----------
            for kb in range(4):
                xin = xin2_0 if kb == 0 else load_x_block(X2, kb, 0)
                xt2, x28 = transpose_block(xin, 0, kb)
                for tp in range(4):          # V projection, bf16
                    t = kb * 4 + tp
                    mm = psV.tile([P, D], f32, tag="pv")
                    for cj in range(NDC):
                        nc.tensor.matmul(mm[:], xt2[:, cj, ts(tp, P)],
                                         wv_b[:, cj, :], start=(cj == 0),
                                         stop=(cj == NDC - 1))
                    nc.vector.tensor_add(vt[t][:], mm[:], bv_bcast[:])
                for ci in range(NDC):        # K^T projection, fp8 DoubleRow
                    mm = psV.tile([P, D], f32, tag="pv")
                    for j in range(2):
                        nc.tensor.matmul(mm[:], w8k[j][:, :, ts(ci, P)],
                                         x28[j][:], start=(j == 0),
                                         stop=(j == 1), perf_mode=DR)
                    nc.vector.tensor_scalar_add(
                        k8[ci // 2][:, ci % 2, ts(kb, 512)], mm[:], bk_t[ci])

            # ---------- phase A2+B per q-block ----------
            def prep_q(qb):
                xin = load_x_block(X1, qb, 1)
                _, x18 = transpose_block(xin, 1, qb)
                q8 = [q8p.tile([P, 2, 512], f8, tag=f"q8_{j}",
                               name=f"q8_{j}_{qb}") for j in range(2)]
                for ci in range(NDC):
                    mm = psV.tile([P, D], f32, tag="pv")
                    for j in range(2):
                        nc.tensor.matmul(mm[:], w8q[j][:, :, ts(ci, P)],
                                         x18[j][:], start=(j == 0),
                                         stop=(j == 1), perf_mode=DR)
                    nc.vector.tensor_scalar_add(q8[ci // 2][:, ci % 2, :],
                                                mm[:], bq_t[ci])
                return q8

            qt_next = prep_q(0)
            for qb in range(NQB):
                q8 = qt_next

                pts = []
                dps = psD.tile([2, 512], f32, tag="d")
                for t in range(NKT):
                    smm = psS.tile([P, 512], f32, tag="ps")
                    for j in range(2):
                        nc.tensor.matmul(smm[:], k8[j][:, :, ts(t, P)],
                                         q8[j][:], start=(j == 0),
                                         stop=(j == 1), perf_mode=DR)
                    ptile = ptp.tile([P, 512], bf16, tag="pt")
                    nc.scalar.activation(ptile[:], smm[:], Exp, scale=SCALE)
                    pts.append(ptile)
                    nc.tensor.matmul(dps[:], ones2_b[:], ptile[:],
                                     start=(t == 0), stop=(t == NKT - 1))

                # bounce den row through DRAM to get per-partition columns
                den_sb = cpool.tile([1, 512], f32, tag="den_sb",
                                    name=f"den_sb_{qb}")
                nc.vector.tensor_copy(den_sb[:], dps[0:1, :])
                nc.scalar.dma_start(DEN[qb].unsqueeze(0), den_sb[:])
                den_cols = obuf.tile([P, 4], f32, tag="den_cols")
                for s in range(4):
                    nc.scalar.dma_start(den_cols[:, s:s + 1],
                                        DEN[qb, ts(s, P)].unsqueeze(1))
                rec = obuf.tile([P, 4], f32, tag="rec")
                nc.vector.reciprocal(rec[:], den_cols[:])

                if qb + 1 < NQB:
                    qt_next = prep_q(qb + 1)

                for s in range(4):
                    ops = psO.tile([P, 512], f32, tag="o")
                    for t in range(NKT):
                        nc.tensor.matmul(ops[:], pts[t][:, ts(s, P)],
                                         vt[t][:], start=(t == 0),
                                         stop=(t == NKT - 1))
                    osb = obuf.tile([P, 512], f32, tag="osb")
                    nc.vector.tensor_scalar_mul(osb[:], ops[:],
                                                rec[:, s:s + 1])
                    nc.sync.dma_start(OUT[ts(qb * 4 + s, P), :], osb[:])

    nc.compile()
    return nc


def _get_nc():
    if "nc" not in _CACHE:
        _CACHE["nc"] = _build()
    return _CACHE["nc"]


def kernel(x_1, x_2, Wq, bq, Wk, bk, Wv, bv, **_run_kwargs):
    x_1 = np.ascontiguousarray(np.asarray(x_1, dtype=np.float32))
    x_2 = np.ascontiguousarray(np.asarray(x_2, dtype=np.float32))
    Wq = np.ascontiguousarray(np.asarray(Wq, dtype=np.float32))
    bq = np.ascontiguousarray(np.asarray(bq, dtype=np.float32))
    Wk = np.ascontiguousarray(np.asarray(Wk, dtype=np.float32))
    bk = np.ascontiguousarray(np.asarray(bk, dtype=np.float32))
    Wv = np.ascontiguousarray(np.asarray(Wv, dtype=np.float32))
    bv = np.ascontiguousarray(np.asarray(bv, dtype=np.float32))

    s = np.float32(W8S)
    bpack = np.concatenate([(s * bq).reshape(4, P).T,
                            (s * bk).reshape(4, P).T],
                           axis=1).astype(np.float32)
    bpack = np.ascontiguousarray(bpack)

    nc = _get_nc()
    in_maps = [
        {"x1": x_1[c], "x2": x_2[c], "wq": Wq, "wk": Wk, "wv": Wv,
         "bv": bv, "bpack": bpack}
        for c in range(NCORES)
    ]
    res = run_bass_kernel_spmd(nc, in_maps, list(range(NCORES)),
                               **_run_kwargs)
    if _run_kwargs:
        _CACHE["last_results"] = res
    return np.stack([res.results[c]["out"] for c in range(NCORES)])
